# revision 45
# baseline (speedup 1.0000x reference)
"""MLA (Multi-head Latent Attention) Bass/Tile kernel for 8 Trainium2 NeuronCores.

Problem: nn_MultiHeadLatentAttention_81707457839331
  B=2, S=2048, HID=2048, NH=16 heads, NOPE=128, ROPE=64, VD=128, QKD=192,
  KVR=512, QR=1536, fp32 in/out.

Sharding (single NEFF, SPMD on 8 cores), v2:
  core c -> token slice (batch c//4, tokens [512*(c%4), +512)) for the
  sequence-parallel down-projection, and heads {2c, 2c+1} x BOTH batches
  for kv_up/attention/o_proj.

  All collectives are 8-core (the 4-rank subgroup collectives of v1 were
  the slow path: 43us + 94us):
    - kv path: RMSNorm + k-rope applied LOCALLY (own 512 tokens) before an
      8-core AllGather of the 5 normalized latent chunks (655KB/rank,
      Shared output).
    - q path: q_up for ALL 16 heads on own tokens (identical FLOPs to
      head-sharded q_up on gathered latents), rope+norm pre-comm, then one
      8-core AllToAll (1.57MB/rank) delivers the final assembled qT: the
      old phase C disappears.
  Each core emits o_proj partials for both batches [2, S, HID]; the host
  sums the 8 partials per batch.

Attention (phase D) restructure vs v1:
  - softmax denominators accumulate in PSUM via ones-vector matmuls per key
    chunk (replaces 132us of DVE eaccs adds),
  - 1/den via nc.vector.reciprocal (removes the LN/EXP ACT_TABLE_LOAD
    thrash + the 4.8us x16 PE head-of-line stalls of v1),
  - reciprocal broadcast via gpsimd.partition_broadcast, off the PE queue,
  - query-tile-outer loop, head-pairs processed together,
  - rope score matmuls are K=64 row-tiled (kr in rows 0:64 for even heads,
    64:128 for odd) and run concurrently on the PE array.
"""

import numpy as np
import ml_dtypes

import concourse.bass as bass
import concourse.mybir as mybir
import concourse.tile as tile
from concourse import bacc
from concourse.bass import ds, ts
from concourse.bass_utils import run_bass_kernel_spmd

F32 = mybir.dt.float32
F32R = mybir.dt.float32r
BF16 = mybir.dt.bfloat16
AF = mybir.ActivationFunctionType
NPBF = ml_dtypes.bfloat16

B, S, HID, NH = 2, 2048, 2048, 16
NOPE, ROPE, VD = 128, 64, 128
QKD = NOPE + ROPE
KVR, QR = 512, 1536
EPS = 1e-6
SCALE = QKD ** (-0.5)
P = 128

NHC = HID // P            # 16 hidden chunks
NQC = QR // P             # 12 q-latent chunks
NKC = KVR // P            # 4 ckv chunks
NFC = NKC + 1 + NQC       # 17 down-proj chunks: 4 ckv + [kr|kr] + 12 q
NCH = 3                   # q chunks per target core: nope0, nope1, ropepair
S2 = 2 * S                # both batches' tokens, concatenated
NT8 = 8                   # 512-token tiles across both batches
RG8 = [[0, 1, 2, 3, 4, 5, 6, 7]]
DEBUG = False


def _emit(tc):
    nc = tc.nc
    hid_in = nc.dram_tensor("hid", [P, NHC, 512], BF16, kind="ExternalInput").ap()
    cos_in = nc.dram_tensor("cos2", [P, 512], BF16, kind="ExternalInput").ap()
    sin_in = nc.dram_tensor("sin2", [P, 512], BF16, kind="ExternalInput").ap()
    wd_in = nc.dram_tensor("wd", [NFC, P, NHC, P], BF16, kind="ExternalInput").ap()
    wqup_in = nc.dram_tensor("wqup", [P, NQC, 8 * NCH * P], BF16,
                             kind="ExternalInput").ap()
    wkup_in = nc.dram_tensor("wkup", [P, NKC, 256], BF16, kind="ExternalInput").ap()
    wvup_in = nc.dram_tensor("wvup", [P, NKC, 256], BF16, kind="ExternalInput").ap()
    wo_in = nc.dram_tensor("wo", [P, 2, HID], BF16, kind="ExternalInput").ap()
    prot_in = nc.dram_tensor("prot", [P, P], BF16, kind="ExternalInput").ap()
    out_d = nc.dram_tensor("out", [B, S, HID], BF16, kind="ExternalOutput").ap()
    if DEBUG:
        dbg_qt = nc.dram_tensor("dbg_qt", [P, NCH, S2], BF16,
                                kind="ExternalOutput").ap()
        dbg_kt = nc.dram_tensor("dbg_kt", [P, 2, S2], BF16,
                                kind="ExternalOutput").ap()
        dbg_krp = nc.dram_tensor("dbg_krp", [P, S2], BF16,
                                 kind="ExternalOutput").ap()
        dbg_v = nc.dram_tensor("dbg_v", [P, 32, 256], BF16,
                               kind="ExternalOutput").ap()
        dbg_o = nc.dram_tensor("dbg_o", [P, 2, 2, S], BF16,
                               kind="ExternalOutput").ap()

    with (
        tc.tile_pool(name="const", bufs=1) as constp,
        tc.tile_pool(name="dram", bufs=1, space="DRAM") as dramp,
        tc.tile_pool(name="persist", bufs=1) as pp,
    ):
        eps_kv = constp.tile([1, 1], F32)
        nc.vector.memset(eps_kv, EPS)
        eps_q = constp.tile([1, 1], F32)
        nc.vector.memset(eps_q, EPS / (SCALE * SCALE))
        ones_f32 = constp.tile([P, 1], F32)
        nc.vector.memset(ones_f32, 1.0)
        ones_kf = constp.tile([P, 1], F32R)       # colsum stationary (f32r)
        nc.vector.tensor_copy(ones_kf, ones_f32)
        ones_b = constp.tile([1, P], F32R)        # rank-1 row broadcast
        ones_bf32 = constp.tile([1, P], F32)
        nc.vector.memset(ones_bf32, 1.0)
        nc.vector.tensor_copy(ones_b, ones_bf32)
        # causal 0/1 mask for the narrowed diagonal: mask0[p,x] = x >= p
        # (gpsimd, but dep-free so it runs at t~0 before the collectives)
        mask0 = constp.tile([P, P], BF16, name="mask0")
        nc.gpsimd.memset(mask0, 1.0)
        nc.gpsimd.affine_select(
            out=mask0, in_=mask0, pattern=[[1, P]],
            compare_op=mybir.AluOpType.is_ge, fill=0.0,
            base=0, channel_multiplier=-1,
        )
        # den-packing stationaries: sel4[:, i, :97-32i] routes a colsum into
        # PSUM partition 96-32i (32-aligned for the reciprocal read). Width
        # shrinks with i so tile i's den row is never touched by later
        # query tiles once it finalizes.
        sel4 = constp.tile([P, 4, 97], BF16, name="sel4")
        nc.vector.memset(sel4, 0.0)
        for i in range(4):
            nc.vector.memset(sel4[:, i, 96 - 32 * i:97 - 32 * i], 1.0)

        # collective buffers
        ag_in = dramp.tile([P, 5, 512], BF16)
        ag_out = dramp.tile([NT8, P, 5, 512], BF16, addr_space="Shared")
        a2a_in = dramp.tile([NT8, P, NCH, 512], BF16)
        a2a_out = dramp.tile([NT8, P, NCH, 512], BF16)
        dumm_in = dramp.tile([1, 16], BF16)
        dumm_out = dramp.tile([NT8, 16], BF16, addr_space="Shared")

        # tiny day-0 collective: absorbs the one-time cross-core barrier +
        # first-collective setup (~50us) while phase A computes
        dumm_row = constp.tile([1, 16], BF16)
        nc.vector.memset(dumm_row, 0.0)
        nc.sync.dma_start(dumm_in, dumm_row)
        nc.gpsimd.collective_compute(
            "AllGather", mybir.AluOpType.bypass, RG8,
            ins=[dumm_in[:].opt()], outs=[dumm_out[:].opt()],
        )

        # persistent SBUF across phases
        cos_sb = pp.tile([P, 512], BF16)
        sin_sb = pp.tile([P, 512], BF16)
        nc.sync.dma_start(cos_sb, cos_in)
        nc.sync.dma_start(sin_sb, sin_in)
        prot_sb = pp.tile([P, P], BF16)
        nc.sync.dma_start(prot_sb, prot_in)

        # ---------------- Phase A: seq-parallel down-proj + local norm ----
        with (
            tc.tile_pool(name="paw", bufs=1) as paw,
            tc.tile_pool(name="pa_q", bufs=1) as paq,
            tc.tile_pool(name="pa_row", bufs=3) as par,
            tc.tile_pool(name="pa_t", bufs=2) as pat,
            tc.tile_pool(name="pa_s", bufs=1) as pas,
            tc.tile_pool(name="pa_ps", bufs=4, space="PSUM") as pap,
            tc.tile_pool(name="pa_st", bufs=2, space="PSUM") as past,
            tc.tile_pool(name="pa_bc", bufs=2, space="PSUM") as pabc,
        ):
            with nc.named_scope("phaseA"):
                hid_sb = paw.tile([P, NHC, 512], BF16)
                wd_sb = paw.tile([P, NFC, NHC, P], BF16)
                wq_sb = paw.tile([P, NQC, 8 * NCH * P], BF16)
                nc.sync.dma_start(hid_sb[:, 0:4, :], hid_in[:, 0:4, :])
                nc.sync.dma_start(wd_sb[:, 0], wd_in[0])
                nc.sync.dma_start(hid_sb[:, 4:, :], hid_in[:, 4:, :])
                for fc in range(1, NFC):
                    nc.sync.dma_start(wd_sb[:, fc], wd_in[fc])
                for fc in range(NQC):
                    nc.sync.dma_start(wq_sb[:, fc, :], wqup_in[:, fc, :])

                latq = paq.tile([P, NQC, 512], BF16)
                ckv_rows = paq.tile([P, NKC, 512], BF16)
                sq_acc_kv = paq.tile([P, 512], F32R, name="sqkv")
                sq_acc_q = paq.tile([P, 512], F32R, name="sqq")
                bcs_kv = paq.tile([P, 512], BF16, name="bcskv")
                bcs_q = paq.tile([P, 512], BF16, name="bcsq")

                for fc in range(NFC):
                    ps = pap.tile([P, 512], F32, name="aps")
                    for hc in range(NHC):
                        nc.tensor.matmul(
                            ps, wd_sb[:, fc, hc, :], hid_sb[:, hc, :],
                            start=(hc == 0), stop=(hc == NHC - 1),
                        )
                    if fc < NKC:
                        # normalized later; keep bf16 row + square-accumulate
                        nc.vector.tensor_copy(ckv_rows[:, fc, :], ps)
                        if fc == 0:
                            nc.scalar.square(sq_acc_kv, ckv_rows[:, 0, :])
                        else:
                            sq = pat.tile([P, 512], F32, name="asq")
                            nc.scalar.square(sq, ckv_rows[:, fc, :])
                            nc.vector.tensor_add(sq_acc_kv, sq_acc_kv, sq)
                    elif fc == NKC:
                        # k-rope chunk [kr|kr]: rope now, no norm
                        kr = par.tile([P, 512], BF16, name="aqr")
                        nc.vector.tensor_copy(kr, ps)
                        rt = pabc.tile([P, 512], F32, name="abc")
                        nc.tensor.matmul(rt, prot_sb, kr, start=True, stop=True)
                        t1 = pat.tile([P, 512], F32, name="at1", bufs=1)
                        nc.vector.tensor_mul(t1, kr, cos_sb)
                        t2 = pat.tile([P, 512], F32, name="at2", bufs=1)
                        nc.vector.tensor_mul(t2, rt, sin_sb)
                        nc.vector.tensor_add(t1, t1, t2)
                        krow = par.tile([P, 512], BF16, name="arow")
                        nc.vector.tensor_copy(krow, t1)
                        nc.sync.dma_start(ag_in[:, 4, :], krow)
                    else:
                        qc = fc - NKC - 1
                        nc.vector.tensor_copy(latq[:, qc, :], ps)
                        if qc == 0:
                            nc.scalar.square(sq_acc_q, latq[:, 0, :])
                        else:
                            sq = pat.tile([P, 512], F32, name="asq")
                            nc.scalar.square(sq, latq[:, qc, :])
                            nc.vector.tensor_add(sq_acc_q, sq_acc_q, sq)
                    if fc == NKC - 1:
                        # kv norm scale: colsum -> sqrt -> recip -> broadcast
                        st = past.tile([1, 512], F32, name="ast")
                        nc.tensor.matmul(st, ones_kf, sq_acc_kv,
                                         start=True, stop=True)
                        srt = pas.tile([1, 512], F32, name="asrt")
                        nc.scalar.activation(srt, st, AF.Sqrt, bias=eps_kv,
                                             scale=1.0 / KVR)
                        rk = pas.tile([1, 512], F32, name="ark")
                        nc.vector.reciprocal(rk, srt)
                        rkr = pas.tile([1, 512], F32R, name="arkr")
                        nc.vector.tensor_copy(rkr, rk)
                        bc = pabc.tile([P, 512], F32, name="abc")
                        nc.tensor.matmul(bc, ones_b, rkr, start=True, stop=True)
                        nc.vector.tensor_copy(bcs_kv, bc)
                        for kc in range(NKC):
                            row = par.tile([P, 512], BF16, name="arow")
                            nc.vector.tensor_mul(row, ckv_rows[:, kc, :],
                                                 bcs_kv)
                            nc.sync.dma_start(ag_in[:, kc, :], row)
                    if fc == NFC - 1:
                        st = past.tile([1, 512], F32, name="ast")
                        nc.tensor.matmul(st, ones_kf, sq_acc_q,
                                         start=True, stop=True)
                        srt = pas.tile([1, 512], F32, name="asrt")
                        nc.scalar.activation(srt, st, AF.Sqrt, bias=eps_q,
                                             scale=1.0 / (QR * SCALE * SCALE))
                        rk = pas.tile([1, 512], F32, name="ark")
                        nc.vector.reciprocal(rk, srt)
                        rkr = pas.tile([1, 512], F32R, name="arkr")
                        nc.vector.tensor_copy(rkr, rk)
                        bc = pabc.tile([P, 512], F32, name="abc")
                        nc.tensor.matmul(bc, ones_b, rkr, start=True, stop=True)
                        nc.vector.tensor_copy(bcs_q, bc)
                    if fc == NKC:
                        nc.gpsimd.collective_compute(
                            "AllGather", mybir.AluOpType.bypass, RG8,
                            ins=[ag_in[:].opt()], outs=[ag_out[:].opt()],
                        )

                # ---- Phase A2: q_up for all 16 heads on own tokens ----
                # rope prot-matmuls issue one chunk-group behind so the PE
                # never waits on the DVE qr copy; a2a_in rows go out on the
                # scalar queue so the sync queue's kvn loads aren't blocked
                def flush_rope(qr, j):
                    rt = pabc.tile([P, 512], F32, name="abc")
                    nc.tensor.matmul(rt, prot_sb, qr, start=True, stop=True)
                    t1 = pat.tile([P, 512], F32, name="at1", bufs=1)
                    nc.vector.tensor_mul(t1, qr, cos_sb)
                    t2 = pat.tile([P, 512], F32, name="at2", bufs=1)
                    nc.vector.tensor_mul(t2, rt, sin_sb)
                    nc.vector.tensor_add(t1, t1, t2)
                    row = par.tile([P, 512], BF16, name="arow")
                    nc.vector.tensor_mul(row, t1, bcs_q)
                    nc.scalar.dma_start(a2a_in[j, :, 2, :], row)

                pend_rope = None
                for j in range(NT8):
                    for ch in range(NCH):
                        ps = pap.tile([P, 512], F32, name="aps")
                        col = (j * NCH + ch) * P
                        for fc in range(NQC):
                            nc.tensor.matmul(
                                ps, wq_sb[:, fc, ds(col, P)], latq[:, fc, :],
                                start=(fc == 0), stop=(fc == NQC - 1),
                            )
                        if pend_rope is not None:
                            flush_rope(*pend_rope)
                            pend_rope = None
                        if ch < 2:
                            row = par.tile([P, 512], BF16, name="arow")
                            nc.vector.tensor_mul(row, ps, bcs_q)
                            nc.scalar.dma_start(a2a_in[j, :, ch, :], row)
                        else:
                            qr = par.tile([P, 512], BF16, name="aqr")
                            nc.vector.tensor_copy(qr, ps)
                            pend_rope = (qr, j)
                flush_rope(*pend_rope)
                nc.gpsimd.collective_compute(
                    "AllToAll", mybir.AluOpType.bypass, RG8,
                    ins=[a2a_in[:].opt()], outs=[a2a_out[:].opt()],
                )

        # kt/krp/v/qT stay in SBUF through phases B-D
        with tc.tile_pool(name="pkv", bufs=1) as pkv:
            kt_sb = pkv.tile([P, 2, S2], BF16)     # 2 heads k_nope.T
            kvk = pkv.tile([P, 5, S2], BF16)       # 4 ckv chunks + roped k
            v_sb = pkv.tile([P, 32, 256], BF16)    # V in [token, 2*VD]
            qT = pkv.tile([P, NCH, S2], BF16)      # nope0, nope1, rope-pair

            # ---------------- Phase B: kv_up (no norm needed) ------------
            with (
                tc.tile_pool(name="pb", bufs=1) as pb,
                tc.tile_pool(name="pb_ps", bufs=1, space="PSUM") as pbps,
            ):
                with nc.named_scope("phaseB"):
                    # weights first: tiny, no deps, must not queue behind the
                    # AG-gated kvn loads
                    wk_sb = pb.tile([P, NKC, 256], BF16)
                    nc.sync.dma_start(wk_sb, wkup_in)
                    wv_sb = pb.tile([P, NKC, 256], BF16)
                    nc.sync.dma_start(wv_sb, wvup_in)
                    wo_sb = pkv.tile([P, 2, HID], BF16)
                    nc.sync.dma_start(wo_sb, wo_in)
                    for j in range(NT8):
                        nc.sync.dma_start(kvk[:, :, ts(j, 512)], ag_out[j])
                    # k-heads: stationary reused over 4-tile token groups
                    for d in range(2):
                        for g in range(2):
                            pss = [pbps.tile([P, 512], F32, name="bkp",
                                             bufs=4)
                                   for _ in range(4)]
                            for fc in range(NKC):
                                for tl in range(4):
                                    tt = g * 4 + tl
                                    nc.tensor.matmul(
                                        pss[tl], wk_sb[:, fc, ds(d * P, P)],
                                        kvk[:, fc, ts(tt, 512)],
                                        start=(fc == 0),
                                        stop=(fc == NKC - 1),
                                    )
                            for tl in range(4):
                                nc.vector.tensor_copy(
                                    kt_sb[:, d, ts(g * 4 + tl, 512)], pss[tl])
                    # V in [token, 2*vd]
                    for tch in range(32):
                        ps = pbps.tile([P, 256], F32, name="bvp", bufs=3)
                        for fc in range(NKC):
                            nc.tensor.matmul(
                                ps, kvk[:, fc, ds(tch * P, P)],
                                wv_sb[:, fc, :],
                                start=(fc == 0), stop=(fc == NKC - 1),
                            )
                        nc.vector.tensor_copy(v_sb[:, tch, :], ps)

            # ---------------- Phase C-lite: load qT from A2A -------------
            # one DMA per token slice (dst [P, ch, 512] / src [P, ch, 512]
            # iterate in the same axis order)
            with nc.named_scope("phaseC"):
                for j in range(NT8):
                    nc.sync.dma_start(qT[:, :, ts(j, 512)], a2a_out[j])

            if DEBUG:
                nc.sync.dma_start(dbg_qt, qT)
                nc.sync.dma_start(dbg_kt, kt_sb)
                nc.sync.dma_start(dbg_krp, kvk[:, 4, :])
                nc.sync.dma_start(dbg_v, v_sb)

            # ---------------- Phase D: attention -------------------------
            with tc.tile_pool(name="po", bufs=1) as po:
                o_sb = po.tile([P, 2, 2, S], BF16)   # [vd, h, b, tok]
                with (
                    tc.tile_pool(name="pd_e", bufs=16) as pde,
                    tc.tile_pool(name="pd_t", bufs=6) as pdt,
                    tc.tile_pool(name="pd_r", bufs=6) as pdr,
                    tc.tile_pool(name="pd_sc", bufs=3, space="PSUM") as pdsc,
                    tc.tile_pool(name="pd_o", bufs=4, space="PSUM") as pdo,
                    tc.tile_pool(name="pd_den", bufs=1, space="PSUM") as pdd,
                ):
                    with nc.named_scope("phaseD"):
                        # one (batch, head) instance at a time; key chunks
                        # outer, query tiles in pairs so every stationary
                        # (kt / krp / v chunk) is reused across the pair and
                        # LDWEIGHTS stays off the critical path. AV + den
                        # trail two units behind the scores (softmax exp
                        # latency fully hidden). All 4 query tiles' softmax
                        # denominators accumulate in ONE PSUM bank at
                        # partitions 0/32/64/96 via the sel4 selector.
                        for b in range(2):
                            toff = b * S
                            for h in range(2):
                                rb = 64 * h
                                ps_o = [pdo.tile([P, 512], F32, name="pso")
                                        for _ in range(4)]
                                dens = pdd.tile([97, 512], F32, name="pden")
                                mul_q = []

                                def drain_muls():
                                    # the o_sb mul waits on the gpsimd
                                    # broadcast; emitting it a unit late
                                    # keeps the vector FIFO (masks, recips)
                                    # from blocking behind gpsimd
                                    while mul_q:
                                        i, recb = mul_q.pop(0)
                                        nc.vector.tensor_mul(
                                            o_sb[:, h, b, ts(i, 512)],
                                            ps_o[i], recb)

                                def finalize(i):
                                    rec = pdt.tile([1, 512], F32,
                                                   name="drec")
                                    nc.vector.reciprocal(
                                        rec, dens[ds(96 - 32 * i, 1), :])
                                    recf = pdt.tile([1, 512], BF16,
                                                    name="drecf")
                                    nc.vector.tensor_copy(recf, rec)
                                    recb = pdr.tile([P, 512], BF16,
                                                    name="drecb")
                                    nc.gpsimd.partition_broadcast(recb, recf)
                                    mul_q.append((i, recb))

                                def issue_av_den(jc, unit):
                                    # one LDW of the v chunk covers the
                                    # whole query-tile group
                                    for i, et, qoff, w in unit:
                                        nc.tensor.matmul(
                                            ps_o[i][:, ds(qoff, w)],
                                            v_sb[:, (toff // P) + jc,
                                                 ds(h * P, P)],
                                            et[:, :w],
                                            start=(jc == 0),
                                            stop=(jc == 4 * i + 3))
                                    for i, et, qoff, w in unit:
                                        nwid = 97 - 32 * i
                                        nc.tensor.matmul(
                                            dens[ds(0, nwid), ds(qoff, w)],
                                            sel4[:, i, ds(0, nwid)],
                                            et[:, :w],
                                            start=(jc == 0 and i == 0),
                                            stop=(jc == 4 * i + 3))
                                    hit = [i for i, _e, _q, _w in unit
                                           if jc == 4 * i + 3]
                                    drain_muls()
                                    for i in hit:
                                        finalize(i)

                                pend = []
                                for jc in range(16):
                                    imin = jc // 4
                                    for pi in range(imin, 4, 2):
                                        idxs = [x for x in (pi, pi + 1)
                                                if x < 4]
                                        unit = []
                                        # kt pass: stationary reused
                                        for i in idxs:
                                            m = jc - 4 * i
                                            qoff = max(0, m * P)
                                            w = 512 - qoff
                                            qpos = toff + i * 512 + qoff
                                            ps_sc = pdsc.tile(
                                                [P, 512], F32, name="psc")
                                            nc.tensor.matmul(
                                                ps_sc[:, :w],
                                                kt_sb[:, h,
                                                      ds(toff + jc * P, P)],
                                                qT[:, h, ds(qpos, w)],
                                                start=True, stop=False)
                                            unit.append((i, ps_sc, qoff, w))
                                        # krp pass: stationary reused
                                        unit2 = []
                                        for i, ps_sc, qoff, w in unit:
                                            qpos = toff + i * 512 + qoff
                                            nc.tensor.matmul(
                                                ps_sc[:, :w],
                                                kvk[ds(rb, 64), 4,
                                                    ds(toff + jc * P, P)],
                                                qT[ds(rb, 64), 2,
                                                   ds(qpos, w)],
                                                start=False, stop=True)
                                            et = pde.tile([P, 512], BF16,
                                                          name="et")
                                            nc.scalar.activation(
                                                et[:, :w], ps_sc[:, :w],
                                                AF.Exp)
                                            if jc - 4 * i >= 0:
                                                nc.vector.tensor_mul(
                                                    et[:, :P], et[:, :P],
                                                    mask0)
                                            unit2.append((i, et, qoff, w))
                                        pend.append((jc, unit2))
                                        if len(pend) > 2:
                                            issue_av_den(*pend.pop(0))
                                for p in pend:
                                    issue_av_den(*p)
                                drain_muls()

                if DEBUG:
                    nc.sync.dma_start(dbg_o, o_sb)

                # ---------------- Phase F: o_proj partials ---------------
                with (
                    tc.tile_pool(name="pf", bufs=1) as pf,
                    tc.tile_pool(name="pf_r", bufs=3) as pfr,
                    tc.tile_pool(name="pf_ps", bufs=4, space="PSUM") as pfp,
                ):
                    with nc.named_scope("phaseF"):
                        for b in range(2):
                            for tch in range(16):
                                orow = pfr.tile([P, HID], BF16, name="orow")
                                pss = [pfp.tile([P, 512], F32, name="fps")
                                       for _ in range(4)]
                                for h in range(2):
                                    for ct in range(4):
                                        nc.tensor.matmul(
                                            pss[ct],
                                            o_sb[:, h, b, ds(tch * P, P)],
                                            wo_sb[:, h, ts(ct, 512)],
                                            start=(h == 0), stop=(h == 1),
                                        )
                                for ct in range(4):
                                    nc.vector.tensor_copy(
                                        orow[:, ts(ct, 512)], pss[ct])
                                nc.sync.dma_start(
                                    out_d[b, ds(tch * P, P), :], orow)


_NC_CACHE = None


def _build_nc():
    global _NC_CACHE
    if _NC_CACHE is None:
        nc = bacc.Bacc("TRN2", target_bir_lowering=False, debug=False,
                       num_devices=8)
        with tile.TileContext(nc) as tc:
            _emit(tc)
        nc.compile()
        _NC_CACHE = nc
    return _NC_CACHE


def _shard_inputs(hidden_states, cos, sin, Wq_down, q_gamma, Wq_up,
                  Wkv_down, kv_gamma, Wkv_up, Wo):
    f32 = np.float32
    hid = np.asarray(hidden_states, dtype=f32)
    cos = np.asarray(cos, dtype=f32)
    sin = np.asarray(sin, dtype=f32)
    Wqd = np.asarray(Wq_down, dtype=f32)
    Wkd = np.asarray(Wkv_down, dtype=f32)
    qg = np.asarray(q_gamma, dtype=f32)
    kvg = np.asarray(kv_gamma, dtype=f32)
    Wqu = np.asarray(Wq_up, dtype=f32) * qg[None, :]
    Wku = np.asarray(Wkv_up, dtype=f32) * kvg[None, :]
    Wo = np.asarray(Wo, dtype=f32)

    # combined down-proj weight: [ckv(512) | kr|kr(128) | q(1536)]
    WckvT = Wkd[:KVR].T                            # [HID, KVR]
    krope = Wkd[KVR:].T                            # [HID, 64]
    WdT = np.concatenate([WckvT, krope, krope, Wqd.T], 1)   # [HID, 2176]
    wd = np.ascontiguousarray(
        WdT.reshape(NHC, P, NFC, P).transpose(2, 1, 0, 3)).astype(NPBF)

    # rotate_half permutation for the 64-dim rope blocks (both halves)
    prot = np.zeros((P, P), dtype=f32)
    for base in (0, 64):
        for t in range(32):
            prot[base + 32 + t, base + t] = -1.0
            prot[base + t, base + 32 + t] = 1.0
    prot = prot.astype(NPBF)

    # q_up for ALL heads; chunk order per target core j: nope2j, nope2j+1, pair
    cols = []
    for j in range(8):
        h0, h1 = 2 * j, 2 * j + 1
        b0 = Wqu[h0 * QKD:(h0 + 1) * QKD]
        b1 = Wqu[h1 * QKD:(h1 + 1) * QKD]
        cols += [b0[:NOPE], b1[:NOPE],
                 np.concatenate([b0[NOPE:], b1[NOPE:]], 0)]
    WquT = np.concatenate(cols, 0).T               # [QR, 3072]
    wqup = np.ascontiguousarray(
        WquT.reshape(NQC, P, 8 * NCH * P).transpose(1, 0, 2)).astype(NPBF)

    in_maps = []
    for c in range(8):
        b, g = c // 4, c % 4
        sl = slice(g * 512, (g + 1) * 512)
        h_sw = np.ascontiguousarray(
            hid[b].T.reshape(NHC, P, S).transpose(1, 0, 2)[:, :, sl]
        ).astype(NPBF)
        cT, sT = cos[b].T[:, sl], sin[b].T[:, sl]
        cos2 = np.ascontiguousarray(np.concatenate([cT, cT], 0)).astype(NPBF)
        sin2 = np.ascontiguousarray(np.concatenate([sT, sT], 0)).astype(NPBF)

        h0, h1 = 2 * c, 2 * c + 1
        kb, vb = [], []
        for h in (h0, h1):
            blk = Wku[h * (NOPE + VD):(h + 1) * (NOPE + VD)]
            kb.append(blk[:NOPE])
            vb.append(blk[NOPE:])
        WkuT = np.concatenate(kb, 0).T             # [KVR, 256]
        WvuT = np.concatenate(vb, 0).T
        wkup = np.ascontiguousarray(
            WkuT.reshape(NKC, P, 256).transpose(1, 0, 2)).astype(NPBF)
        wvup = np.ascontiguousarray(
            WvuT.reshape(NKC, P, 256).transpose(1, 0, 2)).astype(NPBF)
        wo = np.ascontiguousarray(np.stack(
            [Wo[:, h * VD:(h + 1) * VD].T for h in (h0, h1)], 1)).astype(NPBF)

        in_maps.append({
            "hid": h_sw, "cos2": cos2, "sin2": sin2, "wd": wd, "prot": prot,
            "wqup": wqup, "wkup": wkup, "wvup": wvup, "wo": wo,
        })
    return in_maps


def kernel(hidden_states, cos, sin, Wq_down, q_gamma, Wq_up,
           Wkv_down, kv_gamma, Wkv_up, Wo, _trace=False):
    nc = _build_nc()
    in_maps = _shard_inputs(hidden_states, cos, sin, Wq_down, q_gamma, Wq_up,
                            Wkv_down, kv_gamma, Wkv_up, Wo)
    res = run_bass_kernel_spmd(nc, in_maps, core_ids=list(range(8)),
                               trace=_trace)
    out = np.zeros((B, S, HID), dtype=np.float32)
    for c in range(8):
        out += np.asarray(res.results[c]["out"], dtype=np.float32)
    if _trace:
        kernel.last_results = res
    return out


# revision 46
# speedup vs baseline: 1.1042x; 1.1042x over previous
"""MLA (Multi-head Latent Attention) Bass/Tile kernel for 8 Trainium2 NeuronCores.

Problem: nn_MultiHeadLatentAttention_81707457839331
  B=2, S=2048, HID=2048, NH=16 heads, NOPE=128, ROPE=64, VD=128, QKD=192,
  KVR=512, QR=1536, fp32 in/out.

Sharding (single NEFF, SPMD on 8 cores), v2:
  core c -> token slice (batch c//4, tokens [512*(c%4), +512)) for the
  sequence-parallel down-projection, and heads {2c, 2c+1} x BOTH batches
  for kv_up/attention/o_proj.

  All collectives are 8-core (the 4-rank subgroup collectives of v1 were
  the slow path: 43us + 94us):
    - kv path: RMSNorm + k-rope applied LOCALLY (own 512 tokens) before an
      8-core AllGather of the 5 normalized latent chunks (655KB/rank,
      Shared output).
    - q path: q_up for ALL 16 heads on own tokens (identical FLOPs to
      head-sharded q_up on gathered latents), rope+norm pre-comm, then one
      8-core AllToAll (1.57MB/rank) delivers the final assembled qT: the
      old phase C disappears.
  Each core emits o_proj partials for both batches [2, S, HID]; the host
  sums the 8 partials per batch.

Attention (phase D) restructure vs v1:
  - softmax denominators accumulate in PSUM via ones-vector matmuls per key
    chunk (replaces 132us of DVE eaccs adds),
  - 1/den via nc.vector.reciprocal (removes the LN/EXP ACT_TABLE_LOAD
    thrash + the 4.8us x16 PE head-of-line stalls of v1),
  - reciprocal broadcast via gpsimd.partition_broadcast, off the PE queue,
  - query-tile-outer loop, head-pairs processed together,
  - rope score matmuls are K=64 row-tiled (kr in rows 0:64 for even heads,
    64:128 for odd) and run concurrently on the PE array.
"""

import numpy as np
import ml_dtypes

import concourse.bass as bass
import concourse.mybir as mybir
import concourse.tile as tile
from concourse import bacc
from concourse.bass import ds, ts
from concourse.bass_utils import run_bass_kernel_spmd

F32 = mybir.dt.float32
F32R = mybir.dt.float32r
BF16 = mybir.dt.bfloat16
AF = mybir.ActivationFunctionType
NPBF = ml_dtypes.bfloat16

B, S, HID, NH = 2, 2048, 2048, 16
NOPE, ROPE, VD = 128, 64, 128
QKD = NOPE + ROPE
KVR, QR = 512, 1536
EPS = 1e-6
SCALE = QKD ** (-0.5)
P = 128

NHC = HID // P            # 16 hidden chunks
NQC = QR // P             # 12 q-latent chunks
NKC = KVR // P            # 4 ckv chunks
NFC = NKC + 1 + NQC       # 17 down-proj chunks: 4 ckv + [kr|kr] + 12 q
NCH = 3                   # q chunks per target core: nope0, nope1, ropepair
S2 = 2 * S                # both batches' tokens, concatenated
NT8 = 8                   # 512-token tiles across both batches
RG8 = [[0, 1, 2, 3, 4, 5, 6, 7]]
DEBUG = False


def _emit(tc):
    nc = tc.nc
    hid_in = nc.dram_tensor("hid", [P, NHC, 512], BF16, kind="ExternalInput").ap()
    cos_in = nc.dram_tensor("cos2", [P, 512], BF16, kind="ExternalInput").ap()
    sin_in = nc.dram_tensor("sin2", [P, 512], BF16, kind="ExternalInput").ap()
    wd_in = nc.dram_tensor("wd", [NFC, P, NHC, P], BF16, kind="ExternalInput").ap()
    wqup_in = nc.dram_tensor("wqup", [P, NQC, 8 * NCH * P], BF16,
                             kind="ExternalInput").ap()
    wkup_in = nc.dram_tensor("wkup", [P, NKC, 256], BF16, kind="ExternalInput").ap()
    wvup_in = nc.dram_tensor("wvup", [P, NKC, 256], BF16, kind="ExternalInput").ap()
    wo_in = nc.dram_tensor("wo", [P, 2, HID], BF16, kind="ExternalInput").ap()
    prot_in = nc.dram_tensor("prot", [P, P], BF16, kind="ExternalInput").ap()
    out_d = nc.dram_tensor("out", [B, S, HID], BF16, kind="ExternalOutput").ap()
    if DEBUG:
        dbg_qt = nc.dram_tensor("dbg_qt", [P, NCH, S2], BF16,
                                kind="ExternalOutput").ap()
        dbg_kt = nc.dram_tensor("dbg_kt", [P, 2, S2], BF16,
                                kind="ExternalOutput").ap()
        dbg_krp = nc.dram_tensor("dbg_krp", [P, S2], BF16,
                                 kind="ExternalOutput").ap()
        dbg_v = nc.dram_tensor("dbg_v", [P, 32, 256], BF16,
                               kind="ExternalOutput").ap()
        dbg_o = nc.dram_tensor("dbg_o", [P, 2, 2, S], BF16,
                               kind="ExternalOutput").ap()

    with (
        tc.tile_pool(name="const", bufs=1) as constp,
        tc.tile_pool(name="dram", bufs=1, space="DRAM") as dramp,
        tc.tile_pool(name="persist", bufs=1) as pp,
    ):
        eps_kv = constp.tile([1, 1], F32)
        nc.vector.memset(eps_kv, EPS)
        eps_q = constp.tile([1, 1], F32)
        nc.vector.memset(eps_q, EPS / (SCALE * SCALE))
        ones_f32 = constp.tile([P, 1], F32)
        nc.vector.memset(ones_f32, 1.0)
        ones_kf = constp.tile([P, 1], F32R)       # colsum stationary (f32r)
        nc.vector.tensor_copy(ones_kf, ones_f32)
        ones_b = constp.tile([1, P], F32R)        # rank-1 row broadcast
        ones_bf32 = constp.tile([1, P], F32)
        nc.vector.memset(ones_bf32, 1.0)
        nc.vector.tensor_copy(ones_b, ones_bf32)
        # causal 0/1 mask for the narrowed diagonal: mask0[p,x] = x >= p
        # (gpsimd, but dep-free so it runs at t~0 before the collectives)
        mask0 = constp.tile([P, P], BF16, name="mask0")
        nc.gpsimd.memset(mask0, 1.0)
        nc.gpsimd.affine_select(
            out=mask0, in_=mask0, pattern=[[1, P]],
            compare_op=mybir.AluOpType.is_ge, fill=0.0,
            base=0, channel_multiplier=-1,
        )
        # den-packing stationaries: sel4[:, i, :97-32i] routes a colsum into
        # PSUM partition 96-32i (32-aligned for the reciprocal read). Width
        # shrinks with i so tile i's den row is never touched by later
        # query tiles once it finalizes.
        sel4 = constp.tile([P, 4, 97], BF16, name="sel4")
        nc.vector.memset(sel4, 0.0)
        for i in range(4):
            nc.vector.memset(sel4[:, i, 96 - 32 * i:97 - 32 * i], 1.0)

        # collective buffers
        ag_in = dramp.tile([P, 5, 512], BF16)
        ag_out = dramp.tile([NT8, P, 5, 512], BF16, addr_space="Shared")
        a2a_in = dramp.tile([NT8, P, NCH, 512], BF16)
        a2a_out = dramp.tile([NT8, P, NCH, 512], BF16)
        dumm_in = dramp.tile([1, 16], BF16)
        dumm_out = dramp.tile([NT8, 16], BF16, addr_space="Shared")

        # tiny day-0 collective: absorbs the one-time cross-core barrier +
        # first-collective setup (~50us) while phase A computes
        dumm_row = constp.tile([1, 16], BF16)
        nc.vector.memset(dumm_row, 0.0)
        nc.sync.dma_start(dumm_in, dumm_row)
        nc.gpsimd.collective_compute(
            "AllGather", mybir.AluOpType.bypass, RG8,
            ins=[dumm_in[:].opt()], outs=[dumm_out[:].opt()],
        )

        # persistent SBUF across phases
        cos_sb = pp.tile([P, 512], BF16)
        sin_sb = pp.tile([P, 512], BF16)
        nc.sync.dma_start(cos_sb, cos_in)
        nc.sync.dma_start(sin_sb, sin_in)
        prot_sb = pp.tile([P, P], BF16)
        nc.sync.dma_start(prot_sb, prot_in)

        # ---------------- Phase A: seq-parallel down-proj + local norm ----
        with (
            tc.tile_pool(name="paw", bufs=1) as paw,
            tc.tile_pool(name="pa_q", bufs=1) as paq,
            tc.tile_pool(name="pa_row", bufs=3) as par,
            tc.tile_pool(name="pa_t", bufs=2) as pat,
            tc.tile_pool(name="pa_s", bufs=1) as pas,
            tc.tile_pool(name="pa_ps", bufs=4, space="PSUM") as pap,
            tc.tile_pool(name="pa_st", bufs=2, space="PSUM") as past,
            tc.tile_pool(name="pa_bc", bufs=2, space="PSUM") as pabc,
        ):
            with nc.named_scope("phaseA"):
                hid_sb = paw.tile([P, NHC, 512], BF16)
                wd_sb = paw.tile([P, NFC, NHC, P], BF16)
                wq_sb = paw.tile([P, NQC, 8 * NCH * P], BF16)
                nc.sync.dma_start(hid_sb[:, 0:4, :], hid_in[:, 0:4, :])
                nc.sync.dma_start(wd_sb[:, 0], wd_in[0])
                nc.sync.dma_start(hid_sb[:, 4:, :], hid_in[:, 4:, :])
                for fc in range(1, NFC):
                    nc.sync.dma_start(wd_sb[:, fc], wd_in[fc])
                for fc in range(NQC):
                    nc.sync.dma_start(wq_sb[:, fc, :], wqup_in[:, fc, :])

                latq = paq.tile([P, NQC, 512], BF16)
                ckv_rows = paq.tile([P, NKC, 512], BF16)
                sq_acc_kv = paq.tile([P, 512], F32R, name="sqkv")
                sq_acc_q = paq.tile([P, 512], F32R, name="sqq")
                bcs_kv = paq.tile([P, 512], BF16, name="bcskv")
                bcs_q = paq.tile([P, 512], BF16, name="bcsq")

                for fc in range(NFC):
                    ps = pap.tile([P, 512], F32, name="aps")
                    for hc in range(NHC):
                        nc.tensor.matmul(
                            ps, wd_sb[:, fc, hc, :], hid_sb[:, hc, :],
                            start=(hc == 0), stop=(hc == NHC - 1),
                        )
                    if fc < NKC:
                        # normalized later; keep bf16 row + square-accumulate
                        nc.vector.tensor_copy(ckv_rows[:, fc, :], ps)
                        if fc == 0:
                            nc.scalar.square(sq_acc_kv, ckv_rows[:, 0, :])
                        else:
                            sq = pat.tile([P, 512], F32, name="asq")
                            nc.scalar.square(sq, ckv_rows[:, fc, :])
                            nc.vector.tensor_add(sq_acc_kv, sq_acc_kv, sq)
                    elif fc == NKC:
                        # k-rope chunk [kr|kr]: rope now, no norm
                        kr = par.tile([P, 512], BF16, name="aqr")
                        nc.vector.tensor_copy(kr, ps)
                        rt = pabc.tile([P, 512], F32, name="abc")
                        nc.tensor.matmul(rt, prot_sb, kr, start=True, stop=True)
                        t1 = pat.tile([P, 512], F32, name="at1", bufs=1)
                        nc.vector.tensor_mul(t1, kr, cos_sb)
                        t2 = pat.tile([P, 512], F32, name="at2", bufs=1)
                        nc.vector.tensor_mul(t2, rt, sin_sb)
                        nc.vector.tensor_add(t1, t1, t2)
                        krow = par.tile([P, 512], BF16, name="arow")
                        nc.vector.tensor_copy(krow, t1)
                        nc.sync.dma_start(ag_in[:, 4, :], krow)
                    else:
                        qc = fc - NKC - 1
                        nc.vector.tensor_copy(latq[:, qc, :], ps)
                        if qc == 0:
                            nc.scalar.square(sq_acc_q, latq[:, 0, :])
                        else:
                            sq = pat.tile([P, 512], F32, name="asq")
                            nc.scalar.square(sq, latq[:, qc, :])
                            nc.vector.tensor_add(sq_acc_q, sq_acc_q, sq)
                    if fc == NKC - 1:
                        # kv norm scale: colsum -> sqrt -> recip -> broadcast
                        st = past.tile([1, 512], F32, name="ast")
                        nc.tensor.matmul(st, ones_kf, sq_acc_kv,
                                         start=True, stop=True)
                        srt = pas.tile([1, 512], F32, name="asrt")
                        nc.scalar.activation(srt, st, AF.Sqrt, bias=eps_kv,
                                             scale=1.0 / KVR)
                        rk = pas.tile([1, 512], F32, name="ark")
                        nc.vector.reciprocal(rk, srt)
                        rkr = pas.tile([1, 512], F32R, name="arkr")
                        nc.vector.tensor_copy(rkr, rk)
                        bc = pabc.tile([P, 512], F32, name="abc")
                        nc.tensor.matmul(bc, ones_b, rkr, start=True, stop=True)
                        nc.vector.tensor_copy(bcs_kv, bc)
                        for kc in range(NKC):
                            row = par.tile([P, 512], BF16, name="arow")
                            nc.vector.tensor_mul(row, ckv_rows[:, kc, :],
                                                 bcs_kv)
                            nc.sync.dma_start(ag_in[:, kc, :], row)
                    if fc == NFC - 1:
                        st = past.tile([1, 512], F32, name="ast")
                        nc.tensor.matmul(st, ones_kf, sq_acc_q,
                                         start=True, stop=True)
                        srt = pas.tile([1, 512], F32, name="asrt")
                        nc.scalar.activation(srt, st, AF.Sqrt, bias=eps_q,
                                             scale=1.0 / (QR * SCALE * SCALE))
                        rk = pas.tile([1, 512], F32, name="ark")
                        nc.vector.reciprocal(rk, srt)
                        rkr = pas.tile([1, 512], F32R, name="arkr")
                        nc.vector.tensor_copy(rkr, rk)
                        bc = pabc.tile([P, 512], F32, name="abc")
                        nc.tensor.matmul(bc, ones_b, rkr, start=True, stop=True)
                        nc.vector.tensor_copy(bcs_q, bc)
                    if fc == NKC:
                        nc.gpsimd.collective_compute(
                            "AllGather", mybir.AluOpType.bypass, RG8,
                            ins=[ag_in[:].opt()], outs=[ag_out[:].opt()],
                        )

                # ---- Phase A2: q_up for all 16 heads on own tokens ----
                # rope prot-matmuls issue one chunk-group behind so the PE
                # never waits on the DVE qr copy; a2a_in rows go out on the
                # scalar queue so the sync queue's kvn loads aren't blocked
                def flush_rope(qr, j):
                    rt = pabc.tile([P, 512], F32, name="abc")
                    nc.tensor.matmul(rt, prot_sb, qr, start=True, stop=True)
                    t1 = pat.tile([P, 512], F32, name="at1", bufs=1)
                    nc.vector.tensor_mul(t1, qr, cos_sb)
                    t2 = pat.tile([P, 512], F32, name="at2", bufs=1)
                    nc.vector.tensor_mul(t2, rt, sin_sb)
                    nc.vector.tensor_add(t1, t1, t2)
                    row = par.tile([P, 512], BF16, name="arow")
                    nc.vector.tensor_mul(row, t1, bcs_q)
                    nc.scalar.dma_start(a2a_in[j, :, 2, :], row)

                pend_rope = None
                for j in range(NT8):
                    for ch in range(NCH):
                        ps = pap.tile([P, 512], F32, name="aps")
                        col = (j * NCH + ch) * P
                        for fc in range(NQC):
                            nc.tensor.matmul(
                                ps, wq_sb[:, fc, ds(col, P)], latq[:, fc, :],
                                start=(fc == 0), stop=(fc == NQC - 1),
                            )
                        if pend_rope is not None:
                            flush_rope(*pend_rope)
                            pend_rope = None
                        if ch < 2:
                            row = par.tile([P, 512], BF16, name="arow")
                            nc.vector.tensor_mul(row, ps, bcs_q)
                            nc.scalar.dma_start(a2a_in[j, :, ch, :], row)
                        else:
                            qr = par.tile([P, 512], BF16, name="aqr")
                            nc.vector.tensor_copy(qr, ps)
                            pend_rope = (qr, j)
                flush_rope(*pend_rope)
                nc.gpsimd.collective_compute(
                    "AllToAll", mybir.AluOpType.bypass, RG8,
                    ins=[a2a_in[:].opt()], outs=[a2a_out[:].opt()],
                )

        # kt/krp/v/qT stay in SBUF through phases B-D
        with tc.tile_pool(name="pkv", bufs=1) as pkv:
            kt_sb = pkv.tile([P, 2, S2], BF16)     # 2 heads k_nope.T
            kvk = pkv.tile([P, 5, S2], BF16)       # 4 ckv chunks + roped k
            v_sb = pkv.tile([P, 32, 256], BF16)    # V in [token, 2*VD]
            qT = pkv.tile([P, NCH, S2], BF16)      # nope0, nope1, rope-pair

            # ---------------- Phase B: kv_up (no norm needed) ------------
            with (
                tc.tile_pool(name="pb", bufs=1) as pb,
                tc.tile_pool(name="pb_ps", bufs=1, space="PSUM") as pbps,
            ):
                with nc.named_scope("phaseB"):
                    # weights first: tiny, no deps, must not queue behind the
                    # AG-gated kvn loads
                    wk_sb = pb.tile([P, NKC, 256], BF16)
                    nc.sync.dma_start(wk_sb, wkup_in)
                    wv_sb = pb.tile([P, NKC, 256], BF16)
                    nc.sync.dma_start(wv_sb, wvup_in)
                    wo_sb = pkv.tile([P, 2, HID], BF16)
                    nc.sync.dma_start(wo_sb, wo_in)
                    for j in range(NT8):
                        nc.sync.dma_start(kvk[:, :, ts(j, 512)], ag_out[j])
                    # k-heads: stationary reused over 4-tile token groups
                    for d in range(2):
                        for g in range(2):
                            pss = [pbps.tile([P, 512], F32, name="bkp",
                                             bufs=4)
                                   for _ in range(4)]
                            for fc in range(NKC):
                                for tl in range(4):
                                    tt = g * 4 + tl
                                    nc.tensor.matmul(
                                        pss[tl], wk_sb[:, fc, ds(d * P, P)],
                                        kvk[:, fc, ts(tt, 512)],
                                        start=(fc == 0),
                                        stop=(fc == NKC - 1),
                                    )
                            for tl in range(4):
                                nc.vector.tensor_copy(
                                    kt_sb[:, d, ts(g * 4 + tl, 512)], pss[tl])
                    # V in [token, 2*vd]
                    for tch in range(32):
                        ps = pbps.tile([P, 256], F32, name="bvp", bufs=3)
                        for fc in range(NKC):
                            nc.tensor.matmul(
                                ps, kvk[:, fc, ds(tch * P, P)],
                                wv_sb[:, fc, :],
                                start=(fc == 0), stop=(fc == NKC - 1),
                            )
                        nc.vector.tensor_copy(v_sb[:, tch, :], ps)

            # ---------------- Phase C-lite: load qT from A2A -------------
            # one DMA per token slice (dst [P, ch, 512] / src [P, ch, 512]
            # iterate in the same axis order)
            with nc.named_scope("phaseC"):
                for j in range(NT8):
                    nc.sync.dma_start(qT[:, :, ts(j, 512)], a2a_out[j])

            if DEBUG:
                nc.sync.dma_start(dbg_qt, qT)
                nc.sync.dma_start(dbg_kt, kt_sb)
                nc.sync.dma_start(dbg_krp, kvk[:, 4, :])
                nc.sync.dma_start(dbg_v, v_sb)

            # ---------------- Phase D: attention -------------------------
            with tc.tile_pool(name="po", bufs=1) as po:
                o_sb = po.tile([P, 2, 2, S], BF16)   # [vd, h, b, tok]
                with (
                    tc.tile_pool(name="pd_e", bufs=16) as pde,
                    tc.tile_pool(name="pd_t", bufs=6) as pdt,
                    tc.tile_pool(name="pd_r", bufs=6) as pdr,
                    tc.tile_pool(name="pd_sc", bufs=3, space="PSUM") as pdsc,
                    tc.tile_pool(name="pd_o", bufs=4, space="PSUM") as pdo,
                    tc.tile_pool(name="pd_den", bufs=1, space="PSUM") as pdd,
                ):
                    with nc.named_scope("phaseD"):
                        # one (batch, head) instance at a time; key chunks
                        # outer, query tiles in pairs so every stationary
                        # (kt / krp / v chunk) is reused across the pair and
                        # LDWEIGHTS stays off the critical path. AV + den
                        # trail two units behind the scores (softmax exp
                        # latency fully hidden). All 4 query tiles' softmax
                        # denominators accumulate in ONE PSUM bank at
                        # partitions 0/32/64/96 via the sel4 selector.
                        for b in range(2):
                            toff = b * S
                            for h in range(2):
                                rb = 64 * h
                                ps_o = [pdo.tile([P, 512], F32, name="pso")
                                        for _ in range(4)]
                                dens = pdd.tile([97, 512], F32, name="pden")
                                mul_q = []

                                def drain_muls():
                                    # the o_sb mul waits on the gpsimd
                                    # broadcast; emitting it a unit late
                                    # keeps the vector FIFO (masks, recips)
                                    # from blocking behind gpsimd
                                    while mul_q:
                                        i, recb = mul_q.pop(0)
                                        nc.vector.tensor_mul(
                                            o_sb[:, h, b, ts(i, 512)],
                                            ps_o[i], recb)

                                def finalize(i):
                                    # stage PSUM row to SBUF: the approx
                                    # reciprocal needs raw fp32 bits, and
                                    # exact reciprocal on [1,512] is a
                                    # 3.4us single-lane crawl
                                    dsb = pdt.tile([1, 512], F32,
                                                   name="ddsb")
                                    nc.vector.tensor_copy(
                                        dsb, dens[ds(96 - 32 * i, 1), :])
                                    rec = pdt.tile([1, 512], F32,
                                                   name="drec")
                                    nc.vector.reciprocal_approx_fast(
                                        rec, dsb)
                                    recf = pdt.tile([1, 512], BF16,
                                                    name="drecf")
                                    nc.vector.tensor_copy(recf, rec)
                                    recb = pdr.tile([P, 512], BF16,
                                                    name="drecb")
                                    nc.gpsimd.partition_broadcast(recb, recf)
                                    mul_q.append((i, recb))

                                def issue_av_den(jc, unit):
                                    # one LDW of the v chunk covers the
                                    # whole query-tile group
                                    for i, et, qoff, w in unit:
                                        nc.tensor.matmul(
                                            ps_o[i][:, ds(qoff, w)],
                                            v_sb[:, (toff // P) + jc,
                                                 ds(h * P, P)],
                                            et[:, :w],
                                            start=(jc == 0),
                                            stop=(jc == 4 * i + 3))
                                    for i, et, qoff, w in unit:
                                        nwid = 97 - 32 * i
                                        nc.tensor.matmul(
                                            dens[ds(0, nwid), ds(qoff, w)],
                                            sel4[:, i, ds(0, nwid)],
                                            et[:, :w],
                                            start=(jc == 0 and i == 0),
                                            stop=(jc == 4 * i + 3))
                                    hit = [i for i, _e, _q, _w in unit
                                           if jc == 4 * i + 3]
                                    drain_muls()
                                    for i in hit:
                                        finalize(i)

                                pend = []
                                for jc in range(16):
                                    imin = jc // 4
                                    for pi in range(imin, 4, 2):
                                        idxs = [x for x in (pi, pi + 1)
                                                if x < 4]
                                        unit = []
                                        # kt pass: stationary reused
                                        for i in idxs:
                                            m = jc - 4 * i
                                            qoff = max(0, m * P)
                                            w = 512 - qoff
                                            qpos = toff + i * 512 + qoff
                                            ps_sc = pdsc.tile(
                                                [P, 512], F32, name="psc")
                                            nc.tensor.matmul(
                                                ps_sc[:, :w],
                                                kt_sb[:, h,
                                                      ds(toff + jc * P, P)],
                                                qT[:, h, ds(qpos, w)],
                                                start=True, stop=False)
                                            unit.append((i, ps_sc, qoff, w))
                                        # krp pass: stationary reused
                                        unit2 = []
                                        for i, ps_sc, qoff, w in unit:
                                            qpos = toff + i * 512 + qoff
                                            nc.tensor.matmul(
                                                ps_sc[:, :w],
                                                kvk[ds(rb, 64), 4,
                                                    ds(toff + jc * P, P)],
                                                qT[ds(rb, 64), 2,
                                                   ds(qpos, w)],
                                                start=False, stop=True)
                                            et = pde.tile([P, 512], BF16,
                                                          name="et")
                                            nc.scalar.activation(
                                                et[:, :w], ps_sc[:, :w],
                                                AF.Exp)
                                            if jc - 4 * i >= 0:
                                                nc.vector.tensor_mul(
                                                    et[:, :P], et[:, :P],
                                                    mask0)
                                            unit2.append((i, et, qoff, w))
                                        pend.append((jc, unit2))
                                        if len(pend) > 2:
                                            issue_av_den(*pend.pop(0))
                                for p in pend:
                                    issue_av_den(*p)
                                drain_muls()

                if DEBUG:
                    nc.sync.dma_start(dbg_o, o_sb)

                # ---------------- Phase F: o_proj partials ---------------
                with (
                    tc.tile_pool(name="pf", bufs=1) as pf,
                    tc.tile_pool(name="pf_r", bufs=3) as pfr,
                    tc.tile_pool(name="pf_ps", bufs=4, space="PSUM") as pfp,
                ):
                    with nc.named_scope("phaseF"):
                        for b in range(2):
                            for tch in range(16):
                                orow = pfr.tile([P, HID], BF16, name="orow")
                                pss = [pfp.tile([P, 512], F32, name="fps")
                                       for _ in range(4)]
                                for h in range(2):
                                    for ct in range(4):
                                        nc.tensor.matmul(
                                            pss[ct],
                                            o_sb[:, h, b, ds(tch * P, P)],
                                            wo_sb[:, h, ts(ct, 512)],
                                            start=(h == 0), stop=(h == 1),
                                        )
                                for ct in range(4):
                                    nc.vector.tensor_copy(
                                        orow[:, ts(ct, 512)], pss[ct])
                                nc.sync.dma_start(
                                    out_d[b, ds(tch * P, P), :], orow)


_NC_CACHE = None


def _build_nc():
    global _NC_CACHE
    if _NC_CACHE is None:
        nc = bacc.Bacc("TRN2", target_bir_lowering=False, debug=False,
                       num_devices=8)
        with tile.TileContext(nc) as tc:
            _emit(tc)
        nc.compile()
        _NC_CACHE = nc
    return _NC_CACHE


def _shard_inputs(hidden_states, cos, sin, Wq_down, q_gamma, Wq_up,
                  Wkv_down, kv_gamma, Wkv_up, Wo):
    f32 = np.float32
    hid = np.asarray(hidden_states, dtype=f32)
    cos = np.asarray(cos, dtype=f32)
    sin = np.asarray(sin, dtype=f32)
    Wqd = np.asarray(Wq_down, dtype=f32)
    Wkd = np.asarray(Wkv_down, dtype=f32)
    qg = np.asarray(q_gamma, dtype=f32)
    kvg = np.asarray(kv_gamma, dtype=f32)
    Wqu = np.asarray(Wq_up, dtype=f32) * qg[None, :]
    Wku = np.asarray(Wkv_up, dtype=f32) * kvg[None, :]
    Wo = np.asarray(Wo, dtype=f32)

    # combined down-proj weight: [ckv(512) | kr|kr(128) | q(1536)]
    WckvT = Wkd[:KVR].T                            # [HID, KVR]
    krope = Wkd[KVR:].T                            # [HID, 64]
    WdT = np.concatenate([WckvT, krope, krope, Wqd.T], 1)   # [HID, 2176]
    wd = np.ascontiguousarray(
        WdT.reshape(NHC, P, NFC, P).transpose(2, 1, 0, 3)).astype(NPBF)

    # rotate_half permutation for the 64-dim rope blocks (both halves)
    prot = np.zeros((P, P), dtype=f32)
    for base in (0, 64):
        for t in range(32):
            prot[base + 32 + t, base + t] = -1.0
            prot[base + t, base + 32 + t] = 1.0
    prot = prot.astype(NPBF)

    # q_up for ALL heads; chunk order per target core j: nope2j, nope2j+1, pair
    cols = []
    for j in range(8):
        h0, h1 = 2 * j, 2 * j + 1
        b0 = Wqu[h0 * QKD:(h0 + 1) * QKD]
        b1 = Wqu[h1 * QKD:(h1 + 1) * QKD]
        cols += [b0[:NOPE], b1[:NOPE],
                 np.concatenate([b0[NOPE:], b1[NOPE:]], 0)]
    WquT = np.concatenate(cols, 0).T               # [QR, 3072]
    wqup = np.ascontiguousarray(
        WquT.reshape(NQC, P, 8 * NCH * P).transpose(1, 0, 2)).astype(NPBF)

    in_maps = []
    for c in range(8):
        b, g = c // 4, c % 4
        sl = slice(g * 512, (g + 1) * 512)
        h_sw = np.ascontiguousarray(
            hid[b].T.reshape(NHC, P, S).transpose(1, 0, 2)[:, :, sl]
        ).astype(NPBF)
        cT, sT = cos[b].T[:, sl], sin[b].T[:, sl]
        cos2 = np.ascontiguousarray(np.concatenate([cT, cT], 0)).astype(NPBF)
        sin2 = np.ascontiguousarray(np.concatenate([sT, sT], 0)).astype(NPBF)

        h0, h1 = 2 * c, 2 * c + 1
        kb, vb = [], []
        for h in (h0, h1):
            blk = Wku[h * (NOPE + VD):(h + 1) * (NOPE + VD)]
            kb.append(blk[:NOPE])
            vb.append(blk[NOPE:])
        WkuT = np.concatenate(kb, 0).T             # [KVR, 256]
        WvuT = np.concatenate(vb, 0).T
        wkup = np.ascontiguousarray(
            WkuT.reshape(NKC, P, 256).transpose(1, 0, 2)).astype(NPBF)
        wvup = np.ascontiguousarray(
            WvuT.reshape(NKC, P, 256).transpose(1, 0, 2)).astype(NPBF)
        wo = np.ascontiguousarray(np.stack(
            [Wo[:, h * VD:(h + 1) * VD].T for h in (h0, h1)], 1)).astype(NPBF)

        in_maps.append({
            "hid": h_sw, "cos2": cos2, "sin2": sin2, "wd": wd, "prot": prot,
            "wqup": wqup, "wkup": wkup, "wvup": wvup, "wo": wo,
        })
    return in_maps


def kernel(hidden_states, cos, sin, Wq_down, q_gamma, Wq_up,
           Wkv_down, kv_gamma, Wkv_up, Wo, _trace=False):
    nc = _build_nc()
    in_maps = _shard_inputs(hidden_states, cos, sin, Wq_down, q_gamma, Wq_up,
                            Wkv_down, kv_gamma, Wkv_up, Wo)
    res = run_bass_kernel_spmd(nc, in_maps, core_ids=list(range(8)),
                               trace=_trace)
    out = np.zeros((B, S, HID), dtype=np.float32)
    for c in range(8):
        out += np.asarray(res.results[c]["out"], dtype=np.float32)
    if _trace:
        kernel.last_results = res
    return out


# revision 49
# speedup vs baseline: 1.1416x; 1.0338x over previous
"""MLA (Multi-head Latent Attention) Bass/Tile kernel for 8 Trainium2 NeuronCores.

Problem: nn_MultiHeadLatentAttention_81707457839331
  B=2, S=2048, HID=2048, NH=16 heads, NOPE=128, ROPE=64, VD=128, QKD=192,
  KVR=512, QR=1536, fp32 in/out.

Sharding (single NEFF, SPMD on 8 cores), v2:
  core c -> token slice (batch c//4, tokens [512*(c%4), +512)) for the
  sequence-parallel down-projection, and heads {2c, 2c+1} x BOTH batches
  for kv_up/attention/o_proj.

  All collectives are 8-core (the 4-rank subgroup collectives of v1 were
  the slow path: 43us + 94us):
    - kv path: RMSNorm + k-rope applied LOCALLY (own 512 tokens) before an
      8-core AllGather of the 5 normalized latent chunks (655KB/rank,
      Shared output).
    - q path: q_up for ALL 16 heads on own tokens (identical FLOPs to
      head-sharded q_up on gathered latents), rope+norm pre-comm, then one
      8-core AllToAll (1.57MB/rank) delivers the final assembled qT: the
      old phase C disappears.
  Each core emits o_proj partials for both batches [2, S, HID]; the host
  sums the 8 partials per batch.

Attention (phase D) restructure vs v1:
  - softmax denominators accumulate in PSUM via ones-vector matmuls per key
    chunk (replaces 132us of DVE eaccs adds),
  - 1/den via nc.vector.reciprocal (removes the LN/EXP ACT_TABLE_LOAD
    thrash + the 4.8us x16 PE head-of-line stalls of v1),
  - reciprocal broadcast via gpsimd.partition_broadcast, off the PE queue,
  - query-tile-outer loop, head-pairs processed together,
  - rope score matmuls are K=64 row-tiled (kr in rows 0:64 for even heads,
    64:128 for odd) and run concurrently on the PE array.
"""

import numpy as np
import ml_dtypes

import concourse.bass as bass
import concourse.mybir as mybir
import concourse.tile as tile
from concourse import bacc
from concourse.bass import ds, ts
from concourse.bass_utils import run_bass_kernel_spmd

F32 = mybir.dt.float32
F32R = mybir.dt.float32r
BF16 = mybir.dt.bfloat16
AF = mybir.ActivationFunctionType
NPBF = ml_dtypes.bfloat16

B, S, HID, NH = 2, 2048, 2048, 16
NOPE, ROPE, VD = 128, 64, 128
QKD = NOPE + ROPE
KVR, QR = 512, 1536
EPS = 1e-6
SCALE = QKD ** (-0.5)
P = 128

NHC = HID // P            # 16 hidden chunks
NQC = QR // P             # 12 q-latent chunks
NKC = KVR // P            # 4 ckv chunks
NFC = NKC + 1 + NQC       # 17 down-proj chunks: 4 ckv + [kr|kr] + 12 q
NCH = 3                   # q chunks per target core: nope0, nope1, ropepair
S2 = 2 * S                # both batches' tokens, concatenated
NT8 = 8                   # 512-token tiles across both batches
RG8 = [[0, 1, 2, 3, 4, 5, 6, 7]]
DEBUG = False


def _emit(tc):
    nc = tc.nc
    hid_in = nc.dram_tensor("hid", [P, NHC, 512], BF16, kind="ExternalInput").ap()
    cos_in = nc.dram_tensor("cos2", [P, 512], BF16, kind="ExternalInput").ap()
    sin_in = nc.dram_tensor("sin2", [P, 512], BF16, kind="ExternalInput").ap()
    wd_in = nc.dram_tensor("wd", [NFC, P, NHC, P], BF16, kind="ExternalInput").ap()
    wqup_in = nc.dram_tensor("wqup", [P, NQC, 8 * NCH * P], BF16,
                             kind="ExternalInput").ap()
    wkup_in = nc.dram_tensor("wkup", [P, NKC, 256], BF16, kind="ExternalInput").ap()
    wvup_in = nc.dram_tensor("wvup", [P, NKC, 256], BF16, kind="ExternalInput").ap()
    wo_in = nc.dram_tensor("wo", [P, 2, HID], BF16, kind="ExternalInput").ap()
    prot_in = nc.dram_tensor("prot", [P, P], BF16, kind="ExternalInput").ap()
    out_d = nc.dram_tensor("out", [B, S, HID], BF16, kind="ExternalOutput").ap()
    if DEBUG:
        dbg_qt = nc.dram_tensor("dbg_qt", [P, NCH, S2], BF16,
                                kind="ExternalOutput").ap()
        dbg_kt = nc.dram_tensor("dbg_kt", [P, 2, S2], BF16,
                                kind="ExternalOutput").ap()
        dbg_krp = nc.dram_tensor("dbg_krp", [P, S2], BF16,
                                 kind="ExternalOutput").ap()
        dbg_v = nc.dram_tensor("dbg_v", [P, 32, 256], BF16,
                               kind="ExternalOutput").ap()
        dbg_o = nc.dram_tensor("dbg_o", [P, 2, 2, S], BF16,
                               kind="ExternalOutput").ap()

    with (
        tc.tile_pool(name="const", bufs=1) as constp,
        tc.tile_pool(name="dram", bufs=1, space="DRAM") as dramp,
        tc.tile_pool(name="persist", bufs=1) as pp,
    ):
        eps_kv = constp.tile([1, 1], F32)
        nc.vector.memset(eps_kv, EPS)
        eps_q = constp.tile([1, 1], F32)
        nc.vector.memset(eps_q, EPS / (SCALE * SCALE))
        ones_f32 = constp.tile([P, 1], F32)
        nc.vector.memset(ones_f32, 1.0)
        ones_kf = constp.tile([P, 1], F32R)       # colsum stationary (f32r)
        nc.vector.tensor_copy(ones_kf, ones_f32)
        ones_b = constp.tile([1, P], F32R)        # rank-1 row broadcast
        ones_bf32 = constp.tile([1, P], F32)
        nc.vector.memset(ones_bf32, 1.0)
        nc.vector.tensor_copy(ones_b, ones_bf32)
        # causal 0/1 mask for the narrowed diagonal: mask0[p,x] = x >= p
        # (gpsimd, but dep-free so it runs at t~0 before the collectives)
        mask0 = constp.tile([P, P], BF16, name="mask0")
        nc.gpsimd.memset(mask0, 1.0)
        nc.gpsimd.affine_select(
            out=mask0, in_=mask0, pattern=[[1, P]],
            compare_op=mybir.AluOpType.is_ge, fill=0.0,
            base=0, channel_multiplier=-1,
        )
        # den-packing stationaries: sel4[:, i, :97-32i] routes a colsum into
        # PSUM partition 96-32i (32-aligned for the reciprocal read). Width
        # shrinks with i so tile i's den row is never touched by later
        # query tiles once it finalizes.
        sel4 = constp.tile([P, 4, 97], BF16, name="sel4")
        nc.vector.memset(sel4, 0.0)
        for i in range(4):
            nc.vector.memset(sel4[:, i, 96 - 32 * i:97 - 32 * i], 1.0)

        # collective buffers
        ag_in = dramp.tile([P, 5, 512], BF16)
        ag_out = dramp.tile([NT8, P, 5, 512], BF16, addr_space="Shared")
        a2a_in = dramp.tile([NT8, P, NCH, 512], BF16)
        a2a_out = dramp.tile([NT8, P, NCH, 512], BF16)
        dumm_in = dramp.tile([1, 16], BF16)
        dumm_out = dramp.tile([NT8, 16], BF16, addr_space="Shared")

        # tiny day-0 collective: absorbs the one-time cross-core barrier +
        # first-collective setup (~50us) while phase A computes
        dumm_row = constp.tile([1, 16], BF16)
        nc.vector.memset(dumm_row, 0.0)
        nc.sync.dma_start(dumm_in, dumm_row)
        nc.gpsimd.collective_compute(
            "AllGather", mybir.AluOpType.bypass, RG8,
            ins=[dumm_in[:].opt()], outs=[dumm_out[:].opt()],
        )

        # persistent SBUF across phases
        cos_sb = pp.tile([P, 512], BF16)
        sin_sb = pp.tile([P, 512], BF16)
        nc.sync.dma_start(cos_sb, cos_in)
        nc.sync.dma_start(sin_sb, sin_in)
        prot_sb = pp.tile([P, P], BF16)
        nc.sync.dma_start(prot_sb, prot_in)

        # ---------------- Phase A: seq-parallel down-proj + local norm ----
        with (
            tc.tile_pool(name="paw", bufs=1) as paw,
            tc.tile_pool(name="pa_q", bufs=1) as paq,
            tc.tile_pool(name="pa_row", bufs=3) as par,
            tc.tile_pool(name="pa_t", bufs=2) as pat,
            tc.tile_pool(name="pa_s", bufs=1) as pas,
            tc.tile_pool(name="pa_ps", bufs=4, space="PSUM") as pap,
            tc.tile_pool(name="pa_st", bufs=2, space="PSUM") as past,
            tc.tile_pool(name="pa_bc", bufs=2, space="PSUM") as pabc,
        ):
            with nc.named_scope("phaseA"):
                hid_sb = paw.tile([P, NHC, 512], BF16)
                wd_sb = paw.tile([P, NFC, NHC, P], BF16)
                wq_sb = paw.tile([P, NQC, 8 * NCH * P], BF16)
                nc.sync.dma_start(hid_sb[:, 0:4, :], hid_in[:, 0:4, :])
                nc.sync.dma_start(wd_sb[:, 0], wd_in[0])
                nc.sync.dma_start(hid_sb[:, 4:, :], hid_in[:, 4:, :])
                for fc in range(1, NFC):
                    nc.sync.dma_start(wd_sb[:, fc], wd_in[fc])
                for fc in range(NQC):
                    nc.sync.dma_start(wq_sb[:, fc, :], wqup_in[:, fc, :])

                latq = paq.tile([P, NQC, 512], BF16)
                ckv_rows = paq.tile([P, NKC, 512], BF16)
                sq_acc_kv = paq.tile([P, 512], F32R, name="sqkv")
                sq_acc_q = paq.tile([P, 512], F32R, name="sqq")
                bcs_kv = paq.tile([P, 512], BF16, name="bcskv")
                bcs_q = paq.tile([P, 512], BF16, name="bcsq")

                for fc in range(NFC):
                    ps = pap.tile([P, 512], F32, name="aps")
                    for hc in range(NHC):
                        nc.tensor.matmul(
                            ps, wd_sb[:, fc, hc, :], hid_sb[:, hc, :],
                            start=(hc == 0), stop=(hc == NHC - 1),
                        )
                    if fc < NKC:
                        # normalized later; keep bf16 row + square-accumulate
                        nc.vector.tensor_copy(ckv_rows[:, fc, :], ps)
                        if fc == 0:
                            nc.scalar.square(sq_acc_kv, ckv_rows[:, 0, :])
                        else:
                            sq = pat.tile([P, 512], F32, name="asq")
                            nc.scalar.square(sq, ckv_rows[:, fc, :])
                            nc.vector.tensor_add(sq_acc_kv, sq_acc_kv, sq)
                    elif fc == NKC:
                        # k-rope chunk [kr|kr]: rope now, no norm
                        kr = par.tile([P, 512], BF16, name="aqr")
                        nc.vector.tensor_copy(kr, ps)
                        rt = pabc.tile([P, 512], F32, name="abc")
                        nc.tensor.matmul(rt, prot_sb, kr, start=True, stop=True)
                        t1 = pat.tile([P, 512], F32, name="at1", bufs=1)
                        nc.vector.tensor_mul(t1, kr, cos_sb)
                        t2 = pat.tile([P, 512], F32, name="at2", bufs=1)
                        nc.vector.tensor_mul(t2, rt, sin_sb)
                        nc.vector.tensor_add(t1, t1, t2)
                        krow = par.tile([P, 512], BF16, name="arow")
                        nc.vector.tensor_copy(krow, t1)
                        nc.sync.dma_start(ag_in[:, 4, :], krow)
                    else:
                        qc = fc - NKC - 1
                        nc.vector.tensor_copy(latq[:, qc, :], ps)
                        if qc == 0:
                            nc.scalar.square(sq_acc_q, latq[:, 0, :])
                        else:
                            sq = pat.tile([P, 512], F32, name="asq")
                            nc.scalar.square(sq, latq[:, qc, :])
                            nc.vector.tensor_add(sq_acc_q, sq_acc_q, sq)
                    if fc == NKC - 1:
                        # kv norm scale: colsum -> sqrt -> recip -> broadcast
                        st = past.tile([1, 512], F32, name="ast")
                        nc.tensor.matmul(st, ones_kf, sq_acc_kv,
                                         start=True, stop=True)
                        srt = pas.tile([1, 512], F32, name="asrt")
                        nc.scalar.activation(srt, st, AF.Sqrt, bias=eps_kv,
                                             scale=1.0 / KVR)
                        rk = pas.tile([1, 512], F32, name="ark")
                        nc.vector.reciprocal(rk, srt)
                        rkr = pas.tile([1, 512], F32R, name="arkr")
                        nc.vector.tensor_copy(rkr, rk)
                        bc = pabc.tile([P, 512], F32, name="abc")
                        nc.tensor.matmul(bc, ones_b, rkr, start=True, stop=True)
                        nc.vector.tensor_copy(bcs_kv, bc)
                        for kc in range(NKC):
                            row = par.tile([P, 512], BF16, name="arow")
                            nc.vector.tensor_mul(row, ckv_rows[:, kc, :],
                                                 bcs_kv)
                            nc.sync.dma_start(ag_in[:, kc, :], row)
                    if fc == NFC - 1:
                        st = past.tile([1, 512], F32, name="ast")
                        nc.tensor.matmul(st, ones_kf, sq_acc_q,
                                         start=True, stop=True)
                        srt = pas.tile([1, 512], F32, name="asrt")
                        nc.scalar.activation(srt, st, AF.Sqrt, bias=eps_q,
                                             scale=1.0 / (QR * SCALE * SCALE))
                        rk = pas.tile([1, 512], F32, name="ark")
                        nc.vector.reciprocal(rk, srt)
                        rkr = pas.tile([1, 512], F32R, name="arkr")
                        nc.vector.tensor_copy(rkr, rk)
                        bc = pabc.tile([P, 512], F32, name="abc")
                        nc.tensor.matmul(bc, ones_b, rkr, start=True, stop=True)
                        nc.vector.tensor_copy(bcs_q, bc)
                    if fc == NKC:
                        nc.gpsimd.collective_compute(
                            "AllGather", mybir.AluOpType.bypass, RG8,
                            ins=[ag_in[:].opt()], outs=[ag_out[:].opt()],
                        )

                # ---- Phase A2: q_up for all 16 heads on own tokens ----
                # rope prot-matmuls issue one chunk-group behind so the PE
                # never waits on the DVE qr copy; a2a_in rows go out on the
                # scalar queue so the sync queue's kvn loads aren't blocked
                def flush_rope(qr, j):
                    rt = pabc.tile([P, 512], F32, name="abc")
                    nc.tensor.matmul(rt, prot_sb, qr, start=True, stop=True)
                    t1 = pat.tile([P, 512], F32, name="at1", bufs=1)
                    nc.vector.tensor_mul(t1, qr, cos_sb)
                    t2 = pat.tile([P, 512], F32, name="at2", bufs=1)
                    nc.vector.tensor_mul(t2, rt, sin_sb)
                    nc.vector.tensor_add(t1, t1, t2)
                    row = par.tile([P, 512], BF16, name="arow")
                    nc.vector.tensor_mul(row, t1, bcs_q)
                    nc.scalar.dma_start(a2a_in[j, :, 2, :], row)

                # rope chunk first within each j: its multi-op DVE tail then
                # overlaps the following nope groups, and the final PE
                # instruction of phase A is never a DVE-gated prot matmul
                pend_rope = None
                for j in range(NT8):
                    for ch in (2, 0, 1):
                        ps = pap.tile([P, 512], F32, name="aps")
                        col = (j * NCH + ch) * P
                        for fc in range(NQC):
                            nc.tensor.matmul(
                                ps, wq_sb[:, fc, ds(col, P)], latq[:, fc, :],
                                start=(fc == 0), stop=(fc == NQC - 1),
                            )
                        if pend_rope is not None:
                            flush_rope(*pend_rope)
                            pend_rope = None
                        if ch < 2:
                            row = par.tile([P, 512], BF16, name="arow")
                            nc.vector.tensor_mul(row, ps, bcs_q)
                            nc.scalar.dma_start(a2a_in[j, :, ch, :], row)
                        else:
                            qr = par.tile([P, 512], BF16, name="aqr")
                            nc.vector.tensor_copy(qr, ps)
                            pend_rope = (qr, j)
                if pend_rope is not None:
                    flush_rope(*pend_rope)
                nc.gpsimd.collective_compute(
                    "AllToAll", mybir.AluOpType.bypass, RG8,
                    ins=[a2a_in[:].opt()], outs=[a2a_out[:].opt()],
                )

        # kt/krp/v/qT stay in SBUF through phases B-D
        with tc.tile_pool(name="pkv", bufs=1) as pkv:
            kt_sb = pkv.tile([P, 2, S2], BF16)     # 2 heads k_nope.T
            kvk = pkv.tile([P, 5, S2], BF16)       # 4 ckv chunks + roped k
            v_sb = pkv.tile([P, 32, 256], BF16)    # V in [token, 2*VD]
            qT = pkv.tile([P, NCH, S2], BF16)      # nope0, nope1, rope-pair

            # ---------------- Phase B: kv_up (no norm needed) ------------
            with (
                tc.tile_pool(name="pb", bufs=1) as pb,
                tc.tile_pool(name="pb_ps", bufs=1, space="PSUM") as pbps,
            ):
                with nc.named_scope("phaseB"):
                    # weights first: tiny, no deps, must not queue behind the
                    # AG-gated kvn loads
                    wk_sb = pb.tile([P, NKC, 256], BF16)
                    nc.sync.dma_start(wk_sb, wkup_in)
                    wv_sb = pb.tile([P, NKC, 256], BF16)
                    nc.sync.dma_start(wv_sb, wvup_in)
                    wo_sb = pkv.tile([P, 2, HID], BF16)
                    nc.sync.dma_start(wo_sb, wo_in)
                    for j in range(NT8):
                        nc.sync.dma_start(kvk[:, :, ts(j, 512)], ag_out[j])
                    # k-heads: stationary reused over 4-tile token groups
                    for d in range(2):
                        for g in range(2):
                            pss = [pbps.tile([P, 512], F32, name="bkp",
                                             bufs=4)
                                   for _ in range(4)]
                            for fc in range(NKC):
                                for tl in range(4):
                                    tt = g * 4 + tl
                                    nc.tensor.matmul(
                                        pss[tl], wk_sb[:, fc, ds(d * P, P)],
                                        kvk[:, fc, ts(tt, 512)],
                                        start=(fc == 0),
                                        stop=(fc == NKC - 1),
                                    )
                            for tl in range(4):
                                nc.vector.tensor_copy(
                                    kt_sb[:, d, ts(g * 4 + tl, 512)], pss[tl])
                    # V in [token, 2*vd]
                    for tch in range(32):
                        ps = pbps.tile([P, 256], F32, name="bvp", bufs=3)
                        for fc in range(NKC):
                            nc.tensor.matmul(
                                ps, kvk[:, fc, ds(tch * P, P)],
                                wv_sb[:, fc, :],
                                start=(fc == 0), stop=(fc == NKC - 1),
                            )
                        nc.vector.tensor_copy(v_sb[:, tch, :], ps)

            # ---------------- Phase C-lite: load qT from A2A -------------
            # one DMA per token slice (dst [P, ch, 512] / src [P, ch, 512]
            # iterate in the same axis order)
            with nc.named_scope("phaseC"):
                for j in range(NT8):
                    nc.sync.dma_start(qT[:, :, ts(j, 512)], a2a_out[j])

            if DEBUG:
                nc.sync.dma_start(dbg_qt, qT)
                nc.sync.dma_start(dbg_kt, kt_sb)
                nc.sync.dma_start(dbg_krp, kvk[:, 4, :])
                nc.sync.dma_start(dbg_v, v_sb)

            # ---------------- Phase D: attention -------------------------
            with tc.tile_pool(name="po", bufs=1) as po:
                o_sb = po.tile([P, 2, 2, S], BF16)   # [vd, h, b, tok]
                with (
                    tc.tile_pool(name="pd_e", bufs=16) as pde,
                    tc.tile_pool(name="pd_t", bufs=6) as pdt,
                    tc.tile_pool(name="pd_r", bufs=6) as pdr,
                    tc.tile_pool(name="pd_sc", bufs=3, space="PSUM") as pdsc,
                    tc.tile_pool(name="pd_o", bufs=4, space="PSUM") as pdo,
                    tc.tile_pool(name="pd_den", bufs=1, space="PSUM") as pdd,
                ):
                    with nc.named_scope("phaseD"):
                        # one (batch, head) instance at a time; key chunks
                        # outer, query tiles in pairs so every stationary
                        # (kt / krp / v chunk) is reused across the pair and
                        # LDWEIGHTS stays off the critical path. AV + den
                        # trail two units behind the scores (softmax exp
                        # latency fully hidden). All 4 query tiles' softmax
                        # denominators accumulate in ONE PSUM bank at
                        # partitions 0/32/64/96 via the sel4 selector.
                        for b in range(2):
                            toff = b * S
                            for h in range(2):
                                rb = 64 * h
                                ps_o = [pdo.tile([P, 512], F32, name="pso")
                                        for _ in range(4)]
                                dens = pdd.tile([97, 512], F32, name="pden")
                                mul_q = []

                                def drain_muls():
                                    # the o_sb mul waits on the gpsimd
                                    # broadcast; emitting it a unit late
                                    # keeps the vector FIFO (masks, recips)
                                    # from blocking behind gpsimd
                                    while mul_q:
                                        i, recb = mul_q.pop(0)
                                        nc.vector.tensor_mul(
                                            o_sb[:, h, b, ts(i, 512)],
                                            ps_o[i], recb)

                                def finalize(i):
                                    # stage PSUM row to SBUF: the approx
                                    # reciprocal needs raw fp32 bits, and
                                    # exact reciprocal on [1,512] is a
                                    # 3.4us single-lane crawl
                                    dsb = pdt.tile([1, 512], F32,
                                                   name="ddsb")
                                    nc.vector.tensor_copy(
                                        dsb, dens[ds(96 - 32 * i, 1), :])
                                    rec = pdt.tile([1, 512], F32,
                                                   name="drec")
                                    nc.vector.reciprocal_approx_fast(
                                        rec, dsb)
                                    recf = pdt.tile([1, 512], BF16,
                                                    name="drecf")
                                    nc.vector.tensor_copy(recf, rec)
                                    recb = pdr.tile([P, 512], BF16,
                                                    name="drecb")
                                    nc.gpsimd.partition_broadcast(recb, recf)
                                    mul_q.append((i, recb))

                                def issue_av_den(jc, unit):
                                    # one LDW of the v chunk covers the
                                    # whole query-tile group
                                    for i, et, qoff, w in unit:
                                        nc.tensor.matmul(
                                            ps_o[i][:, ds(qoff, w)],
                                            v_sb[:, (toff // P) + jc,
                                                 ds(h * P, P)],
                                            et[:, :w],
                                            start=(jc == 0),
                                            stop=(jc == 4 * i + 3))
                                    for i, et, qoff, w in unit:
                                        nwid = 97 - 32 * i
                                        nc.tensor.matmul(
                                            dens[ds(0, nwid), ds(qoff, w)],
                                            sel4[:, i, ds(0, nwid)],
                                            et[:, :w],
                                            start=(jc == 0 and i == 0),
                                            stop=(jc == 4 * i + 3))
                                    hit = [i for i, _e, _q, _w in unit
                                           if jc == 4 * i + 3]
                                    drain_muls()
                                    for i in hit:
                                        finalize(i)

                                pend = []
                                for jc in range(16):
                                    imin = jc // 4
                                    for pi in range(imin, 4, 2):
                                        idxs = [x for x in (pi, pi + 1)
                                                if x < 4]
                                        unit = []
                                        # kt pass: stationary reused
                                        for i in idxs:
                                            m = jc - 4 * i
                                            qoff = max(0, m * P)
                                            w = 512 - qoff
                                            qpos = toff + i * 512 + qoff
                                            ps_sc = pdsc.tile(
                                                [P, 512], F32, name="psc")
                                            nc.tensor.matmul(
                                                ps_sc[:, :w],
                                                kt_sb[:, h,
                                                      ds(toff + jc * P, P)],
                                                qT[:, h, ds(qpos, w)],
                                                start=True, stop=False)
                                            unit.append((i, ps_sc, qoff, w))
                                        # krp pass: stationary reused
                                        unit2 = []
                                        for i, ps_sc, qoff, w in unit:
                                            qpos = toff + i * 512 + qoff
                                            nc.tensor.matmul(
                                                ps_sc[:, :w],
                                                kvk[ds(rb, 64), 4,
                                                    ds(toff + jc * P, P)],
                                                qT[ds(rb, 64), 2,
                                                   ds(qpos, w)],
                                                start=False, stop=True)
                                            et = pde.tile([P, 512], BF16,
                                                          name="et")
                                            nc.scalar.activation(
                                                et[:, :w], ps_sc[:, :w],
                                                AF.Exp)
                                            if jc - 4 * i >= 0:
                                                nc.vector.tensor_mul(
                                                    et[:, :P], et[:, :P],
                                                    mask0)
                                            unit2.append((i, et, qoff, w))
                                        pend.append((jc, unit2))
                                        if len(pend) > 2:
                                            issue_av_den(*pend.pop(0))
                                for p in pend:
                                    issue_av_den(*p)
                                drain_muls()

                if DEBUG:
                    nc.sync.dma_start(dbg_o, o_sb)

                # ---------------- Phase F: o_proj partials ---------------
                with (
                    tc.tile_pool(name="pf", bufs=1) as pf,
                    tc.tile_pool(name="pf_r", bufs=3) as pfr,
                    tc.tile_pool(name="pf_ps", bufs=4, space="PSUM") as pfp,
                ):
                    with nc.named_scope("phaseF"):
                        for b in range(2):
                            for tch in range(16):
                                orow = pfr.tile([P, HID], BF16, name="orow")
                                pss = [pfp.tile([P, 512], F32, name="fps")
                                       for _ in range(4)]
                                for h in range(2):
                                    for ct in range(4):
                                        nc.tensor.matmul(
                                            pss[ct],
                                            o_sb[:, h, b, ds(tch * P, P)],
                                            wo_sb[:, h, ts(ct, 512)],
                                            start=(h == 0), stop=(h == 1),
                                        )
                                for ct in range(4):
                                    # split PSUM->bf16 copies across DVE and
                                    # the scalar engine: F is copy-bound on
                                    # a single engine
                                    if ct % 2 == 0:
                                        nc.vector.tensor_copy(
                                            orow[:, ts(ct, 512)], pss[ct])
                                    else:
                                        nc.scalar.copy(
                                            orow[:, ts(ct, 512)], pss[ct])
                                nc.sync.dma_start(
                                    out_d[b, ds(tch * P, P), :], orow)


_NC_CACHE = None


def _build_nc():
    global _NC_CACHE
    if _NC_CACHE is None:
        nc = bacc.Bacc("TRN2", target_bir_lowering=False, debug=False,
                       num_devices=8)
        with tile.TileContext(nc) as tc:
            _emit(tc)
        nc.compile()
        _NC_CACHE = nc
    return _NC_CACHE


def _shard_inputs(hidden_states, cos, sin, Wq_down, q_gamma, Wq_up,
                  Wkv_down, kv_gamma, Wkv_up, Wo):
    f32 = np.float32
    hid = np.asarray(hidden_states, dtype=f32)
    cos = np.asarray(cos, dtype=f32)
    sin = np.asarray(sin, dtype=f32)
    Wqd = np.asarray(Wq_down, dtype=f32)
    Wkd = np.asarray(Wkv_down, dtype=f32)
    qg = np.asarray(q_gamma, dtype=f32)
    kvg = np.asarray(kv_gamma, dtype=f32)
    Wqu = np.asarray(Wq_up, dtype=f32) * qg[None, :]
    Wku = np.asarray(Wkv_up, dtype=f32) * kvg[None, :]
    Wo = np.asarray(Wo, dtype=f32)

    # combined down-proj weight: [ckv(512) | kr|kr(128) | q(1536)]
    WckvT = Wkd[:KVR].T                            # [HID, KVR]
    krope = Wkd[KVR:].T                            # [HID, 64]
    WdT = np.concatenate([WckvT, krope, krope, Wqd.T], 1)   # [HID, 2176]
    wd = np.ascontiguousarray(
        WdT.reshape(NHC, P, NFC, P).transpose(2, 1, 0, 3)).astype(NPBF)

    # rotate_half permutation for the 64-dim rope blocks (both halves)
    prot = np.zeros((P, P), dtype=f32)
    for base in (0, 64):
        for t in range(32):
            prot[base + 32 + t, base + t] = -1.0
            prot[base + t, base + 32 + t] = 1.0
    prot = prot.astype(NPBF)

    # q_up for ALL heads; chunk order per target core j: nope2j, nope2j+1, pair
    cols = []
    for j in range(8):
        h0, h1 = 2 * j, 2 * j + 1
        b0 = Wqu[h0 * QKD:(h0 + 1) * QKD]
        b1 = Wqu[h1 * QKD:(h1 + 1) * QKD]
        cols += [b0[:NOPE], b1[:NOPE],
                 np.concatenate([b0[NOPE:], b1[NOPE:]], 0)]
    WquT = np.concatenate(cols, 0).T               # [QR, 3072]
    wqup = np.ascontiguousarray(
        WquT.reshape(NQC, P, 8 * NCH * P).transpose(1, 0, 2)).astype(NPBF)

    in_maps = []
    for c in range(8):
        b, g = c // 4, c % 4
        sl = slice(g * 512, (g + 1) * 512)
        h_sw = np.ascontiguousarray(
            hid[b].T.reshape(NHC, P, S).transpose(1, 0, 2)[:, :, sl]
        ).astype(NPBF)
        cT, sT = cos[b].T[:, sl], sin[b].T[:, sl]
        cos2 = np.ascontiguousarray(np.concatenate([cT, cT], 0)).astype(NPBF)
        sin2 = np.ascontiguousarray(np.concatenate([sT, sT], 0)).astype(NPBF)

        h0, h1 = 2 * c, 2 * c + 1
        kb, vb = [], []
        for h in (h0, h1):
            blk = Wku[h * (NOPE + VD):(h + 1) * (NOPE + VD)]
            kb.append(blk[:NOPE])
            vb.append(blk[NOPE:])
        WkuT = np.concatenate(kb, 0).T             # [KVR, 256]
        WvuT = np.concatenate(vb, 0).T
        wkup = np.ascontiguousarray(
            WkuT.reshape(NKC, P, 256).transpose(1, 0, 2)).astype(NPBF)
        wvup = np.ascontiguousarray(
            WvuT.reshape(NKC, P, 256).transpose(1, 0, 2)).astype(NPBF)
        wo = np.ascontiguousarray(np.stack(
            [Wo[:, h * VD:(h + 1) * VD].T for h in (h0, h1)], 1)).astype(NPBF)

        in_maps.append({
            "hid": h_sw, "cos2": cos2, "sin2": sin2, "wd": wd, "prot": prot,
            "wqup": wqup, "wkup": wkup, "wvup": wvup, "wo": wo,
        })
    return in_maps


def kernel(hidden_states, cos, sin, Wq_down, q_gamma, Wq_up,
           Wkv_down, kv_gamma, Wkv_up, Wo, _trace=False):
    nc = _build_nc()
    in_maps = _shard_inputs(hidden_states, cos, sin, Wq_down, q_gamma, Wq_up,
                            Wkv_down, kv_gamma, Wkv_up, Wo)
    res = run_bass_kernel_spmd(nc, in_maps, core_ids=list(range(8)),
                               trace=_trace)
    out = np.zeros((B, S, HID), dtype=np.float32)
    for c in range(8):
        out += np.asarray(res.results[c]["out"], dtype=np.float32)
    if _trace:
        kernel.last_results = res
    return out
